# revision 1
# baseline (speedup 1.0000x reference)
"""BEVPool (segment-sum) Trainium2 kernel.

Sharding: Nprime points split contiguously across 8 NeuronCores.
Per core: compute voxel cell per point (reciprocal-multiply floor, bit-exact
vs the jax-on-neuron reference), group cells into "quad rows" (4 cells = one
1KB row so the whole 360x360 grid fits int16 row indexing), deduplicate
quad rows within each 128-token tile via a PE equality-matrix matmul (the
matmul also performs the in-tile aggregation), then dma_scatter_add each
tile's unique rows into round-robin DRAM grids (cross-call ordering is
serialized per grid, so accumulation is exact f32). Host sums partial grids.
"""

import numpy as np

import concourse.bacc as bacc
import concourse.bass as bass
import concourse.mybir as mybir
from concourse import tile
from concourse.bass_utils import run_bass_kernel_spmd

f32 = mybir.dt.float32
i16 = mybir.dt.int16
i32 = mybir.dt.int32
Op = mybir.AluOpType
AX = mybir.AxisListType

NP_TOTAL = 1 * 6 * 118 * 32 * 88          # 1993728 points
NCORES = 8
NP_CORE = NP_TOTAL // NCORES              # 249216 = 128 * 1947
C = 64
H = W = 360
NCELL = H * W                             # 129600
NQUAD = NCELL // 4                        # 32400 quad rows (4 cells each)
GARB = NQUAD                              # garbage quad row
NGRID = 3                                 # round-robin output grids
CHUNK_TILES = 64                          # tiles per chunk (8192 tokens)

RECIP = float(np.float32(np.float32(1.0) / np.float32(0.3)))

_cache = {}


def build_program(np_core=NP_CORE, ncores=NCORES):
    ntiles = np_core // 128
    nc = bacc.Bacc("TRN2", target_bir_lowering=False, debug=False,
                   num_devices=ncores)
    geom_d = nc.dram_tensor("geom", [np_core, 3], f32, kind="ExternalInput")
    x_d = nc.dram_tensor("x", [np_core, C], f32, kind="ExternalInput")
    grids = [
        nc.dram_tensor(f"grid{g}", [NQUAD + 1, 4 * C], f32,
                       kind="ExternalOutput")
        for g in range(NGRID)
    ]

    geom_ap = geom_d.ap()
    x_ap = x_d.ap()

    with tile.TileContext(nc) as tc:
        with (
            tc.tile_pool(name="const", bufs=1) as cpool,
            tc.tile_pool(name="work", bufs=2) as pool,
            tc.tile_pool(name="tiny", bufs=4) as tpool,
            tc.tile_pool(name="psd", bufs=2, space="PSUM") as ppoolD,
            tc.tile_pool(name="psa", bufs=2, space="PSUM") as ppoolA,
        ):
            iota_i = cpool.tile([128, 128], i32, tag="iota_i")
            nc.gpsimd.iota(iota_i[:], [[1, 128]], channel_multiplier=0)
            iota_f = cpool.tile([128, 128], f32, tag="iota_f")
            nc.vector.tensor_copy(iota_f[:], iota_i[:])
            pidx_i = cpool.tile([128, 1], i32, tag="pidx_i")
            nc.gpsimd.iota(pidx_i[:], [[0, 1]], channel_multiplier=1)
            pidx = cpool.tile([128, 1], f32, tag="pidx")
            nc.vector.tensor_copy(pidx[:], pidx_i[:])
            ident = cpool.tile([128, 128], f32, tag="ident")
            nc.vector.tensor_scalar(ident[:], iota_f[:], pidx[:], None,
                                    Op.is_equal)
            ltri = cpool.tile([128, 128], f32, tag="ltri")
            nc.vector.tensor_scalar(ltri[:], iota_f[:], pidx[:], None,
                                    Op.is_lt)
            onesrow = cpool.tile([1, 128], f32, tag="onesrow")
            nc.vector.memset(onesrow[:], 1.0)

            tile_no = 0
            done = 0
            while done < ntiles:
                nt = min(CHUNK_TILES, ntiles - done)
                tok0 = done * 128
                ntok = nt * 128
                # ---- load chunk (token i -> partition i%? : contiguous:
                # partition p holds tokens [p*nt, (p+1)*nt) of the chunk) ----
                xt = pool.tile([128, CHUNK_TILES * C], f32, tag="xt")
                nc.sync.dma_start(
                    xt[:, :nt * C],
                    x_ap[tok0:tok0 + ntok, :].rearrange(
                        "(p t) c -> p (t c)", p=128),
                )
                gt = pool.tile([128, CHUNK_TILES * 3], f32, tag="gt")
                nc.sync.dma_start(
                    gt[:, :nt * 3],
                    geom_ap[tok0:tok0 + ntok, :].rearrange(
                        "(p t) c -> p (t c)", p=128),
                )

                # ---- cell math ----
                def floordiv(coord_ap, tag):
                    w = pool.tile([128, CHUNK_TILES], f32, tag=tag + "w")
                    nc.vector.tensor_scalar(w[:, :nt], coord_ap, 54.0, RECIP,
                                            Op.add, Op.mult)
                    giq = pool.tile([128, CHUNK_TILES], i32, tag=tag + "i")
                    nc.vector.tensor_copy(giq[:, :nt], w[:, :nt])
                    gf = pool.tile([128, CHUNK_TILES], f32, tag=tag + "f")
                    nc.vector.tensor_copy(gf[:, :nt], giq[:, :nt])
                    d = pool.tile([128, CHUNK_TILES], f32, tag=tag + "d")
                    nc.vector.tensor_tensor(d[:, :nt], gf[:, :nt], w[:, :nt],
                                            Op.is_gt)
                    g = pool.tile([128, CHUNK_TILES], f32, tag=tag + "g")
                    nc.vector.tensor_tensor(g[:, :nt], gf[:, :nt], d[:, :nt],
                                            Op.subtract)
                    return g

                gx = floordiv(gt[:, 0:nt * 3:3], "gx")
                gy = floordiv(gt[:, 1:nt * 3:3], "gy")
                cell = pool.tile([128, CHUNK_TILES], f32, tag="cell")
                nc.vector.tensor_scalar(cell[:, :nt], gx[:, :nt], 360.0, None,
                                        Op.mult)
                nc.vector.tensor_tensor(cell[:, :nt], cell[:, :nt],
                                        gy[:, :nt], Op.add)
                nc.vector.tensor_scalar(cell[:, :nt], cell[:, :nt], 0.0,
                                        float(NCELL - 1), Op.max, Op.min)
                quad = pool.tile([128, CHUNK_TILES], f32, tag="quad")
                qi = pool.tile([128, CHUNK_TILES], i32, tag="qi")
                qtrue = pool.tile([128, CHUNK_TILES], f32, tag="qtrue")
                nc.vector.tensor_scalar(qtrue[:, :nt], cell[:, :nt], 0.25,
                                        None, Op.mult)
                nc.vector.tensor_copy(qi[:, :nt], qtrue[:, :nt])
                nc.vector.tensor_copy(quad[:, :nt], qi[:, :nt])
                qd = pool.tile([128, CHUNK_TILES], f32, tag="qd")
                nc.vector.tensor_tensor(qd[:, :nt], quad[:, :nt],
                                        qtrue[:, :nt], Op.is_gt)
                nc.vector.tensor_tensor(quad[:, :nt], quad[:, :nt],
                                        qd[:, :nt], Op.subtract)
                r4 = pool.tile([128, CHUNK_TILES], f32, tag="r4")
                nc.vector.tensor_scalar(r4[:, :nt], quad[:, :nt], -4.0, None,
                                        Op.mult)
                nc.vector.tensor_tensor(r4[:, :nt], r4[:, :nt], cell[:, :nt],
                                        Op.add)
                masks = []
                for s in range(4):
                    m = pool.tile([128, CHUNK_TILES], f32, tag=f"m{s}")
                    nc.vector.tensor_scalar(m[:, :nt], r4[:, :nt], float(s),
                                            None, Op.is_equal)
                    masks.append(m)

                rankarr = pool.tile([128, CHUNK_TILES], f32, tag="rankarr")

                def emat(T, tag):
                    """Equality matrix E[i,j] = (quad_i == quad_j), SBUF."""
                    qcol = quad[:, T:T + 1]
                    psTt = ppoolD.tile([128, 128], f32, tag="psT")
                    psT = psTt[0:1, :]
                    nc.tensor.matmul(psT, qcol, ident[:])
                    qrow = tpool.tile([1, 128], f32, tag="qrow")
                    nc.vector.tensor_copy(qrow[:], psT)
                    nqrow = tpool.tile([1, 128], f32, tag="nqrow")
                    nc.vector.tensor_scalar(nqrow[:], psT, -1.0, None,
                                            Op.mult)
                    psD = ppoolD.tile([128, 128], f32, tag="psD")
                    nc.tensor.matmul(psD[:], qrow[:], onesrow[:],
                                     start=True, stop=False)
                    nc.tensor.matmul(psD[:], onesrow[:], nqrow[:],
                                     start=False, stop=True)
                    E = tpool.tile([128, 128], f32, tag="E")
                    nc.vector.tensor_scalar(E[:], psD[:], 0.0, None,
                                            Op.is_equal)
                    return E

                # ---- phase A: ranks ----
                for T in range(nt):
                    E = emat(T, "a")
                    Elt = tpool.tile([128, 128], f32, tag="Elt")
                    nc.vector.tensor_tensor(Elt[:], E[:], ltri[:], Op.mult)
                    nc.vector.tensor_reduce(rankarr[:, T:T + 1], Elt[:],
                                            AX.X, Op.add)

                # ---- idx select + fold to 16-wrap int16 ----
                idxf = pool.tile([128, CHUNK_TILES], f32, tag="idxf")
                isz = pool.tile([128, CHUNK_TILES], f32, tag="isz")
                nc.vector.tensor_scalar(isz[:, :nt], rankarr[:, :nt], 0.0,
                                        None, Op.is_equal)
                nc.vector.tensor_scalar(idxf[:, :nt], quad[:, :nt],
                                        float(GARB), None, Op.subtract)
                nc.vector.tensor_tensor(idxf[:, :nt], idxf[:, :nt],
                                        isz[:, :nt], Op.mult)
                nc.vector.tensor_scalar(idxf[:, :nt], idxf[:, :nt],
                                        float(GARB), None, Op.add)
                idxt = pool.tile([128, CHUNK_TILES * 8], i16, tag="idxt")
                for r in range(8):
                    nc.gpsimd.dma_start(
                        idxt[0:16, r:8 * nt:8],
                        idxf[16 * r:16 * r + 16, :nt])
                for g8 in range(1, 8):
                    nc.gpsimd.dma_start(
                        idxt[16 * g8:16 * g8 + 16, :8 * nt],
                        idxt[0:16, :8 * nt])

                # ---- phase C: payload, aggregate, scatter ----
                for T in range(nt):
                    E = emat(T, "c")
                    pay = tpool.tile([128, 4 * C], f32, tag="pay")
                    for s in range(4):
                        nc.vector.tensor_scalar(
                            pay[:, s * C:(s + 1) * C],
                            xt[:, T * C:(T + 1) * C],
                            masks[s][:, T:T + 1], None, Op.mult)
                    psA = ppoolA.tile([128, 4 * C], f32, tag="psA")
                    nc.tensor.matmul(psA[:], E[:], pay[:])
                    agg = tpool.tile([128, 4 * C], f32, tag="agg")
                    nc.vector.tensor_copy(agg[:], psA[:])
                    nc.gpsimd.dma_scatter_add(
                        grids[tile_no % NGRID].ap(),
                        agg[:].rearrange("p (b e) -> p b e", b=1),
                        idxt[:, 8 * T:8 * T + 8],
                        128, 128, 4 * C,
                    )
                    tile_no += 1
                done += nt

    nc.compile()
    return nc


def kernel(geom_feats: np.ndarray, x: np.ndarray) -> np.ndarray:
    geom_feats = np.ascontiguousarray(geom_feats, dtype=np.float32)
    x = np.ascontiguousarray(x, dtype=np.float32)
    g2 = geom_feats.reshape(NP_TOTAL, 3)
    x2 = x.reshape(NP_TOTAL, C)

    if "nc" not in _cache:
        _cache["nc"] = build_program()
    nc = _cache["nc"]

    in_maps = []
    for c in range(NCORES):
        sl = slice(c * NP_CORE, (c + 1) * NP_CORE)
        in_maps.append({"geom": g2[sl], "x": x2[sl]})

    res = run_bass_kernel_spmd(nc, in_maps, core_ids=list(range(NCORES)))

    total = np.zeros((NQUAD, 4 * C), np.float64)
    for c in range(NCORES):
        for g in range(NGRID):
            total += res.results[c][f"grid{g}"][:NQUAD].astype(np.float64)
    cells = total.reshape(NQUAD * 4, C).astype(np.float32)
    out = cells.reshape(H, W, C).transpose(2, 0, 1)[None].astype(np.float32)
    return out



# revision 20
# speedup vs baseline: 452.6784x; 452.6784x over previous
"""BEVPool (segment-sum) Trainium2 kernel, v2.

Strategy (8-way data-parallel over points, full local grid per core):
  - Each core gets 1/8 of the 1,993,728 points (249,216 = 1947 tiles of 128).
  - The full 360x360x64 BEV grid lives in SBUF as a bf16 accumulator
    A[128, 32401, 2]: cell = 4*slot + 2*h + e  ->  A[64*h + ch, slot, e]
    (32,400 real quad-slots + 1 dump slot; 126.6 KB per partition).
  - Per 128-token tile: compute the voxel cell (reciprocal-multiply floor,
    matches the jax reference to 1e-6), build a quad-payload [tok, 4*64],
    build the in-tile equality matrix E over quad ids with PE matmuls, and
    compute merged^T = pay_q^T @ E directly with 4 PE matmuls (the merge
    and the transpose fuse). Duplicate tokens (rank>0) are redirected to
    the dump slot; their payload double-lands in the dump row (ignored).
  - One gpsimd.scatter_add per 128-token tile. Call boundaries fully drain
    (WR_drained completion), and in-tile duplicates are merged/redirected,
    so the hardware's lost-update window for same-slot RMWs issued close
    together can never be hit.
  - The accumulator is flushed to DRAM staging 4x per run so bf16 RMW
    chains stay short (hot cells collect ~130 points/core); the final
    sum of the 4 snapshots is f32 on-device.
  - The host sums the 8 cores' f32 grids and reshapes (the "all-reduce"
    of the data-parallel plan).

The in-SBUF gpsimd scatter path replaces the baseline's CCE DMA scatter
(512B quad-row RMWs through one DMA ring), eliminating ~500 MB/core of HBM
read-modify-write traffic.
"""

import numpy as np

import concourse.bacc as bacc
import concourse.bass as bass
import concourse.mybir as mybir
from concourse import tile
from concourse.bass_utils import run_bass_kernel_spmd

f32 = mybir.dt.float32
bf16 = mybir.dt.bfloat16
i16 = mybir.dt.int16
i32 = mybir.dt.int32
Op = mybir.AluOpType
AX = mybir.AxisListType

NP_TOTAL = 1 * 6 * 118 * 32 * 88          # 1993728 points
NCORES = 8
NP_CORE = NP_TOTAL // NCORES              # 249216 = 128 * 1947
C = 64
H = W = 360
NCELL = H * W                             # 129600
NQUAD = NCELL // 4                        # 32400
DUMP = NQUAD                              # dump slot id
NSLOT = NQUAD + 1                         # 32401 accumulator slots
D = 2
CHUNK_TILES = 12                          # tiles per processing chunk
KFLUSH = 4                                # accumulator flushes per run

RECIP = float(np.float32(np.float32(1.0) / np.float32(0.3)))

_cache = {}


def build_program(np_core=NP_CORE, ncores=NCORES):
    ntiles = np_core // 128
    nc = bacc.Bacc("TRN2", target_bir_lowering=False, debug=False,
                   num_devices=ncores)
    geom_d = nc.dram_tensor("geom", [np_core, 3], f32, kind="ExternalInput")
    x_d = nc.dram_tensor("x", [np_core, C], f32, kind="ExternalInput")
    out_d = nc.dram_tensor("out", [128, NSLOT * D], f32,
                           kind="ExternalOutput")
    stage_d = nc.dram_tensor("stage", [KFLUSH, 128, NSLOT * D], bf16,
                             kind="Internal")

    geom_ap = geom_d.ap()
    x_ap = x_d.ap()

    with tile.TileContext(nc) as tc:
        with (
            tc.tile_pool(name="const", bufs=1) as cpool,
            tc.tile_pool(name="acc", bufs=1) as apool,
            tc.tile_pool(name="work", bufs=2) as pool,
            tc.tile_pool(name="etile", bufs=2) as epool,
            tc.tile_pool(name="psd", bufs=2, space="PSUM") as ppoolD,
            tc.tile_pool(name="pst", bufs=1, space="PSUM") as ppoolT,
            tc.tile_pool(name="psm", bufs=1, space="PSUM") as ppoolM,
        ):
            # ---- constants (scratch goes through rotating work tags) ----
            iota_i = pool.tile([128, 256], i32, tag="scrA")
            nc.gpsimd.iota(iota_i[:, :128], [[1, 128]], channel_multiplier=0)
            iota_f = pool.tile([128, 256], f32, tag="scrB")
            nc.vector.tensor_copy(iota_f[:, :128], iota_i[:, :128])
            pidx_i = pool.tile([128, 256], i32, tag="scrA")
            nc.gpsimd.iota(pidx_i[:, :1], [[0, 1]], channel_multiplier=1)
            pidx = pool.tile([128, 256], f32, tag="scrA")
            nc.vector.tensor_copy(pidx[:, :1], pidx_i[:, :1])
            ident = cpool.tile([128, 128], f32, tag="ident")
            nc.vector.tensor_scalar(ident[:], iota_f[:, :128], pidx[:, :1],
                                    None, Op.is_equal)
            ltri = cpool.tile([128, 128], bf16, tag="ltri")
            nc.vector.tensor_scalar(ltri[:], iota_f[:, :128], pidx[:, :1],
                                    None, Op.is_lt)
            onesrow = cpool.tile([1, 128], f32, tag="onesrow")
            nc.vector.memset(onesrow[:], 1.0)
            # quarter-iota row: [0]*64 [1]*64 [2]*64 [3]*64 (floor(j/64))
            io256 = pool.tile([128, 256], i32, tag="scrB")
            nc.gpsimd.iota(io256[:], [[1, 256]], channel_multiplier=0)
            qiof = pool.tile([128, 256], f32, tag="scrA")
            nc.vector.tensor_scalar(qiof[:], io256[:], 1.0 / 64.0, None,
                                    Op.mult)
            qio_i = pool.tile([128, 256], i32, tag="scrB")
            nc.vector.tensor_copy(qio_i[:], qiof[:])
            qiotaf = pool.tile([128, 256], f32, tag="scrA")
            nc.vector.tensor_copy(qiotaf[:], qio_i[:])
            qcorr = pool.tile([128, 256], f32, tag="scrB")
            nc.vector.tensor_tensor(qcorr[:], qiotaf[:], qiof[:], Op.is_gt)
            qiota = cpool.tile([128, 4 * C], f32, tag="qiota")
            nc.vector.tensor_tensor(qiota[:], qiotaf[:], qcorr[:],
                                    Op.subtract)
            ncol_max = 8 * CHUNK_TILES

            # ---- accumulator ----
            A = apool.tile([128, NSLOT * D], bf16, tag="A")
            nc.vector.memset(A[:], 0.0)
            A3 = A[:].rearrange("p (n d) -> p n d", d=D)

            nchunks = (ntiles + CHUNK_TILES - 1) // CHUNK_TILES
            per_flush = (nchunks + KFLUSH - 1) // KFLUSH
            chunk_no = 0
            flush_no = 0
            done = 0
            while done < ntiles:
                nt = min(CHUNK_TILES, ntiles - done)
                tok0 = done * 128
                ntok = nt * 128

                # ---- load ----
                xt = pool.tile([128, CHUNK_TILES * C], f32, tag="xt")
                nc.sync.dma_start(
                    xt[:, :nt * C],
                    x_ap[tok0:tok0 + ntok, :].rearrange(
                        "(p t) c -> p (t c)", p=128))
                gt = pool.tile([128, CHUNK_TILES * 3], f32, tag="gt")
                nc.sync.dma_start(
                    gt[:, :nt * 3],
                    geom_ap[tok0:tok0 + ntok, :].rearrange(
                        "(p t) c -> p (t c)", p=128))

                xbf = pool.tile([128, CHUNK_TILES * C], bf16, tag="xbf")
                nc.vector.tensor_copy(xbf[:, :nt * C], xt[:, :nt * C])

                # ---- cell math (floor via int-convert + correction) ----
                def floordiv(coord_ap, tag):
                    w = pool.tile([128, CHUNK_TILES], f32, tag=tag + "w")
                    nc.vector.tensor_scalar(w[:, :nt], coord_ap, 54.0, RECIP,
                                            Op.add, Op.mult)
                    giq = pool.tile([128, CHUNK_TILES], i32, tag=tag + "i")
                    nc.vector.tensor_copy(giq[:, :nt], w[:, :nt])
                    gf = pool.tile([128, CHUNK_TILES], f32, tag=tag + "f")
                    nc.vector.tensor_copy(gf[:, :nt], giq[:, :nt])
                    d = pool.tile([128, CHUNK_TILES], f32, tag=tag + "d")
                    nc.vector.tensor_tensor(d[:, :nt], gf[:, :nt], w[:, :nt],
                                            Op.is_gt)
                    g = pool.tile([128, CHUNK_TILES], f32, tag=tag + "g")
                    nc.vector.tensor_tensor(g[:, :nt], gf[:, :nt], d[:, :nt],
                                            Op.subtract)
                    return g

                gx = floordiv(gt[:, 0:nt * 3:3], "gx")
                gy = floordiv(gt[:, 1:nt * 3:3], "gy")
                cell = pool.tile([128, CHUNK_TILES], f32, tag="cell")
                nc.vector.tensor_scalar(cell[:, :nt], gx[:, :nt], 360.0, None,
                                        Op.mult)
                nc.vector.tensor_tensor(cell[:, :nt], cell[:, :nt],
                                        gy[:, :nt], Op.add)
                nc.vector.tensor_scalar(cell[:, :nt], cell[:, :nt], 0.0,
                                        float(NCELL - 1), Op.max, Op.min)
                quad = pool.tile([128, CHUNK_TILES], f32, tag="quad")
                qi = pool.tile([128, CHUNK_TILES], i32, tag="qi")
                qtrue = pool.tile([128, CHUNK_TILES], f32, tag="qtrue")
                nc.vector.tensor_scalar(qtrue[:, :nt], cell[:, :nt], 0.25,
                                        None, Op.mult)
                nc.vector.tensor_copy(qi[:, :nt], qtrue[:, :nt])
                nc.vector.tensor_copy(quad[:, :nt], qi[:, :nt])
                qd = pool.tile([128, CHUNK_TILES], f32, tag="qd")
                nc.vector.tensor_tensor(qd[:, :nt], quad[:, :nt],
                                        qtrue[:, :nt], Op.is_gt)
                nc.vector.tensor_tensor(quad[:, :nt], quad[:, :nt],
                                        qd[:, :nt], Op.subtract)
                r4 = pool.tile([128, CHUNK_TILES], f32, tag="r4")
                nc.vector.tensor_scalar(r4[:, :nt], quad[:, :nt], -4.0, None,
                                        Op.mult)
                nc.vector.tensor_tensor(r4[:, :nt], r4[:, :nt], cell[:, :nt],
                                        Op.add)

                # ---- quad payload [tok, 4*64] bf16 ----
                pay = pool.tile([128, CHUNK_TILES * 4 * C], bf16, tag="pay")
                for t in range(nt):
                    m = pool.tile([128, 4 * C], bf16, tag="m256")
                    nc.vector.tensor_scalar(m[:], qiota[:], r4[:, t:t + 1],
                                            None, Op.is_equal)
                    p4 = pay[:, t * 4 * C:(t + 1) * 4 * C].rearrange(
                        "p (q c) -> p q c", c=C)
                    nc.vector.tensor_tensor(
                        p4, m[:].rearrange("p (q c) -> p q c", c=C),
                        xbf[:, t * C:(t + 1) * C].rearrange(
                            "p (u c) -> p u c", u=1).broadcast_to(
                                [128, 4, C]),
                        Op.mult)

                # ---- slab + ranks, per tile ----
                slab = epool.tile([128, CHUNK_TILES * 128 * D], bf16,
                                  tag="slab")
                slab4 = slab[:].rearrange("p (t q d) -> p t q d",
                                          q=128, d=D)
                rankc = pool.tile([128, CHUNK_TILES], f32, tag="rankc")

                for g4 in range((nt + 3) // 4):
                    n4 = min(4, nt - g4 * 4)
                    mt_g = []
                    for q in range(4):
                        mt_q = ppoolM.tile([64, 512], f32, tag=f"mt{q}")
                        mt_g.append(mt_q)
                    for ti in range(n4):
                        t = g4 * 4 + ti
                        # quad^T row for this tile
                        psTt = ppoolT.tile([128, 128], f32, tag="psTt")
                        psT = psTt[0:1, :]
                        nc.tensor.matmul(psT, quad[:, t:t + 1], ident[:])
                        qrow = pool.tile([1, 128], f32, tag="qrow")
                        nc.vector.tensor_copy(qrow[:], psT)
                        nqrow = pool.tile([1, 128], f32, tag="nqrow")
                        nc.vector.tensor_scalar(nqrow[:], psT, -1.0, None,
                                                Op.mult)
                        # E matrix: psD[i,j] = q_i - q_j ; E = (psD == 0)
                        psD = ppoolD.tile([128, 128], f32, tag="psD")
                        nc.tensor.matmul(psD[:], qrow[:], onesrow[:],
                                         start=True, stop=False)
                        nc.tensor.matmul(psD[:], onesrow[:], nqrow[:],
                                         start=False, stop=True)
                        E = epool.tile([128, 128], bf16, tag="E")
                        nc.vector.tensor_scalar(E[:], psD[:], 0.0, None,
                                                Op.is_equal)
                        # rank = count of earlier same-quad tokens
                        escr = pool.tile([128, 128], bf16, tag="escr")
                        nc.vector.tensor_tensor(escr[:], E[:], ltri[:],
                                                Op.mult)
                        nc.vector.tensor_reduce(rankc[:, t:t + 1], escr[:],
                                                AX.X, Op.add)
                        # merged^T per quarter: pay_q^T @ E  (E symmetric)
                        for q in range(4):
                            nc.tensor.matmul(
                                mt_g[q][:, ti * 128:(ti + 1) * 128],
                                pay[:, (t * 4 + q) * C:(t * 4 + q + 1) * C],
                                E[:])
                    # copy merged^T into slab positions
                    for q in range(4):
                        h, e = q >> 1, q & 1
                        nc.vector.tensor_copy(
                            slab4[64 * h:64 * h + 64,
                                  g4 * 4:g4 * 4 + n4, 0:128, e:e + 1],
                            mt_g[q][:, :n4 * 128].rearrange(
                                "p (t s) -> p t s ()", s=128))

                # ---- index selection: rank0 -> quad, else DUMP ----
                isz = pool.tile([128, CHUNK_TILES], f32, tag="isz")
                nc.vector.tensor_scalar(isz[:, :nt], rankc[:, :nt], 0.0,
                                        None, Op.is_equal)
                idxf = pool.tile([128, CHUNK_TILES], f32, tag="idxf")
                nc.vector.tensor_scalar(idxf[:, :nt], quad[:, :nt],
                                        float(DUMP), None, Op.subtract)
                nc.vector.tensor_tensor(idxf[:, :nt], idxf[:, :nt],
                                        isz[:, :nt], Op.mult)
                nc.vector.tensor_scalar(idxf[:, :nt], idxf[:, :nt],
                                        float(DUMP), None, Op.add)
                idx16 = pool.tile([128, CHUNK_TILES], i16, tag="idx16")
                nc.vector.tensor_copy(idx16[:, :nt], idxf[:, :nt])

                # ---- fold idx into wrapped layout [128, 8*nt] ----
                ncol = 8 * nt
                idxw = epool.tile([128, ncol_max], i16, tag="idxw")
                for j in range(8):
                    nc.sync.dma_start(
                        idxw[0:16, j:ncol:8],
                        idx16[16 * j:16 * j + 16, :nt])
                nc.sync.dma_start(idxw[16:32, :ncol], idxw[0:16, :ncol])
                nc.sync.dma_start(idxw[32:64, :ncol], idxw[0:32, :ncol])
                nc.sync.dma_start(idxw[64:128, :ncol], idxw[0:64, :ncol])

                # ---- scatter: one call per tile (call boundaries drain,
                # so only in-tile duplicates matter — and those are merged
                # and redirected to DUMP above) ----
                for t in range(nt):
                    nc.gpsimd.scatter_add(
                        A3,
                        idxw[:, 8 * t:8 * t + 8],
                        slab4[:, t, :, :],
                        128, NSLOT, D, 128)

                done += nt
                chunk_no += 1
                if chunk_no % per_flush == 0 or done >= ntiles:
                    nc.sync.dma_start(stage_d.ap()[flush_no], A[:])
                    if done < ntiles:
                        nc.vector.memset(A[:], 0.0)
                    flush_no += 1

            # ---- final f32 sum of the staged snapshots ----
            Wf = 1024
            per_part = NSLOT * D            # 64802
            off = 0
            while off < per_part:
                w = min(Wf, per_part - off)
                acc = pool.tile([128, Wf], f32, tag="facc")
                st0 = pool.tile([128, Wf], bf16, tag="fst0")
                nc.sync.dma_start(st0[:, :w], stage_d.ap()[0][:, off:off + w])
                nc.vector.tensor_copy(acc[:, :w], st0[:, :w])
                for f in range(1, flush_no):
                    stf = pool.tile([128, Wf], bf16, tag="fstf")
                    nc.sync.dma_start(stf[:, :w],
                                      stage_d.ap()[f][:, off:off + w])
                    nc.vector.tensor_tensor(acc[:, :w], acc[:, :w],
                                            stf[:, :w], Op.add)
                nc.sync.dma_start(out_d.ap()[:, off:off + w], acc[:, :w])
                off += w

    nc.compile()
    return nc


def _decode_grid(arr):
    """[128, NSLOT*D] f32 core dump -> [NCELL, C] f32 grid."""
    a = np.asarray(arr).astype(np.float32).reshape(128, NSLOT, D)
    a = a[:, :NQUAD, :]                       # drop dump slot
    g = a.reshape(2, C, NQUAD, D)             # [h, c, s, e]
    return g.transpose(2, 0, 3, 1).reshape(NCELL, C)


def kernel(geom_feats: np.ndarray, x: np.ndarray) -> np.ndarray:
    geom_feats = np.ascontiguousarray(geom_feats, dtype=np.float32)
    x = np.ascontiguousarray(x, dtype=np.float32)
    g2 = geom_feats.reshape(NP_TOTAL, 3)
    x2 = x.reshape(NP_TOTAL, C)

    if "nc" not in _cache:
        _cache["nc"] = build_program()
    nc = _cache["nc"]

    in_maps = []
    for c in range(NCORES):
        sl = slice(c * NP_CORE, (c + 1) * NP_CORE)
        in_maps.append({"geom": g2[sl], "x": x2[sl]})

    res = run_bass_kernel_spmd(nc, in_maps, core_ids=list(range(NCORES)))

    total = np.zeros((NCELL, C), np.float32)
    for c in range(NCORES):
        total += _decode_grid(res.results[c]["out"])
    out = total.reshape(H, W, C).transpose(2, 0, 1)[None]
    return np.ascontiguousarray(out)


# revision 21
# speedup vs baseline: 455.2898x; 1.0058x over previous
"""BEVPool (segment-sum) Trainium2 kernel, v2.

Strategy (8-way data-parallel over points, full local grid per core):
  - Each core gets 1/8 of the 1,993,728 points (249,216 = 1947 tiles of 128).
  - The full 360x360x64 BEV grid lives in SBUF as a bf16 accumulator
    A[128, 32401, 2]: cell = 4*slot + 2*h + e  ->  A[64*h + ch, slot, e]
    (32,400 real quad-slots + 1 dump slot; 126.6 KB per partition).
  - Per 128-token tile: compute the voxel cell (reciprocal-multiply floor,
    matches the jax reference to 1e-6), build a quad-payload [tok, 4*64],
    build the in-tile equality matrix E over quad ids with PE matmuls, and
    compute merged^T = pay_q^T @ E directly with 4 PE matmuls (the merge
    and the transpose fuse). Duplicate tokens (rank>0) are redirected to
    the dump slot; their payload double-lands in the dump row (ignored).
  - One gpsimd.scatter_add per 128-token tile. Call boundaries fully drain
    (WR_drained completion), and in-tile duplicates are merged/redirected,
    so the hardware's lost-update window for same-slot RMWs issued close
    together can never be hit.
  - The accumulator is flushed to DRAM staging 4x per run so bf16 RMW
    chains stay short (hot cells collect ~130 points/core); the final
    sum of the 4 snapshots is f32 on-device.
  - The host sums the 8 cores' f32 grids and reshapes (the "all-reduce"
    of the data-parallel plan).

The in-SBUF gpsimd scatter path replaces the baseline's CCE DMA scatter
(512B quad-row RMWs through one DMA ring), eliminating ~500 MB/core of HBM
read-modify-write traffic.
"""

import numpy as np

import concourse.bacc as bacc
import concourse.mybir as mybir
from concourse import tile
from concourse.bass_utils import run_bass_kernel_spmd

f32 = mybir.dt.float32
bf16 = mybir.dt.bfloat16
i16 = mybir.dt.int16
i32 = mybir.dt.int32
Op = mybir.AluOpType
AX = mybir.AxisListType

NP_TOTAL = 1 * 6 * 118 * 32 * 88          # 1993728 points
NCORES = 8
NP_CORE = NP_TOTAL // NCORES              # 249216 = 128 * 1947
C = 64
H = W = 360
NCELL = H * W                             # 129600
NQUAD = NCELL // 4                        # 32400
DUMP = NQUAD                              # dump slot id
NSLOT = NQUAD + 1                         # 32401 accumulator slots
D = 2
CHUNK_TILES = 12                          # tiles per processing chunk
KFLUSH = 4                                # accumulator flushes per run

RECIP = float(np.float32(np.float32(1.0) / np.float32(0.3)))

_cache = {}


def build_program(np_core=NP_CORE, ncores=NCORES):
    ntiles = np_core // 128
    nc = bacc.Bacc("TRN2", target_bir_lowering=False, debug=False,
                   num_devices=ncores)
    geom_d = nc.dram_tensor("geom", [np_core, 3], f32, kind="ExternalInput")
    x_d = nc.dram_tensor("x", [np_core, C], f32, kind="ExternalInput")
    out_d = nc.dram_tensor("out", [128, NSLOT * D], f32,
                           kind="ExternalOutput")
    stage_d = nc.dram_tensor("stage", [KFLUSH, 128, NSLOT * D], bf16,
                             kind="Internal")

    geom_ap = geom_d.ap()
    x_ap = x_d.ap()

    with tile.TileContext(nc) as tc:
        with (
            tc.tile_pool(name="const", bufs=1) as cpool,
            tc.tile_pool(name="acc", bufs=1) as apool,
            tc.tile_pool(name="work", bufs=2) as pool,
            tc.tile_pool(name="etile", bufs=2) as epool,
            tc.tile_pool(name="psd", bufs=2, space="PSUM") as ppoolD,
            tc.tile_pool(name="pst", bufs=1, space="PSUM") as ppoolT,
            tc.tile_pool(name="psm", bufs=1, space="PSUM") as ppoolM,
        ):
            # ---- constants (scratch goes through rotating work tags) ----
            iota_i = pool.tile([128, 256], i32, tag="scrA")
            nc.gpsimd.iota(iota_i[:, :128], [[1, 128]], channel_multiplier=0)
            iota_f = pool.tile([128, 256], f32, tag="scrB")
            nc.vector.tensor_copy(iota_f[:, :128], iota_i[:, :128])
            pidx_i = pool.tile([128, 256], i32, tag="scrA")
            nc.gpsimd.iota(pidx_i[:, :1], [[0, 1]], channel_multiplier=1)
            pidx = pool.tile([128, 256], f32, tag="scrA")
            nc.vector.tensor_copy(pidx[:, :1], pidx_i[:, :1])
            ident = cpool.tile([128, 128], f32, tag="ident")
            nc.vector.tensor_scalar(ident[:], iota_f[:, :128], pidx[:, :1],
                                    None, Op.is_equal)
            ltri = cpool.tile([128, 128], bf16, tag="ltri")
            nc.vector.tensor_scalar(ltri[:], iota_f[:, :128], pidx[:, :1],
                                    None, Op.is_lt)
            onesrow = cpool.tile([1, 128], f32, tag="onesrow")
            nc.vector.memset(onesrow[:], 1.0)
            # quarter-iota row: [0]*64 [1]*64 [2]*64 [3]*64 (floor(j/64))
            io256 = pool.tile([128, 256], i32, tag="scrB")
            nc.gpsimd.iota(io256[:], [[1, 256]], channel_multiplier=0)
            qiof = pool.tile([128, 256], f32, tag="scrA")
            nc.vector.tensor_scalar(qiof[:], io256[:], 1.0 / 64.0, None,
                                    Op.mult)
            qio_i = pool.tile([128, 256], i32, tag="scrB")
            nc.vector.tensor_copy(qio_i[:], qiof[:])
            qiotaf = pool.tile([128, 256], f32, tag="scrA")
            nc.vector.tensor_copy(qiotaf[:], qio_i[:])
            qcorr = pool.tile([128, 256], f32, tag="scrB")
            nc.vector.tensor_tensor(qcorr[:], qiotaf[:], qiof[:], Op.is_gt)
            qiota = cpool.tile([128, 4 * C], f32, tag="qiota")
            nc.vector.tensor_tensor(qiota[:], qiotaf[:], qcorr[:],
                                    Op.subtract)
            ncol_max = 8 * CHUNK_TILES

            # ---- accumulator ----
            A = apool.tile([128, NSLOT * D], bf16, tag="A")
            nc.vector.memset(A[:], 0.0)
            A3 = A[:].rearrange("p (n d) -> p n d", d=D)

            nchunks = (ntiles + CHUNK_TILES - 1) // CHUNK_TILES
            per_flush = (nchunks + KFLUSH - 1) // KFLUSH
            chunk_no = 0
            flush_no = 0
            done = 0
            while done < ntiles:
                nt = min(CHUNK_TILES, ntiles - done)
                tok0 = done * 128
                ntok = nt * 128

                # ---- load ----
                xt = pool.tile([128, CHUNK_TILES * C], f32, tag="xt")
                nc.sync.dma_start(
                    xt[:, :nt * C],
                    x_ap[tok0:tok0 + ntok, :].rearrange(
                        "(p t) c -> p (t c)", p=128))
                gt = pool.tile([128, CHUNK_TILES * 3], f32, tag="gt")
                nc.sync.dma_start(
                    gt[:, :nt * 3],
                    geom_ap[tok0:tok0 + ntok, :].rearrange(
                        "(p t) c -> p (t c)", p=128))

                xbf = pool.tile([128, CHUNK_TILES * C], bf16, tag="xbf")
                nc.vector.tensor_copy(xbf[:, :nt * C], xt[:, :nt * C])

                # ---- cell math (floor via int-convert + correction) ----
                def floordiv(coord_ap, tag):
                    w = pool.tile([128, CHUNK_TILES], f32, tag=tag + "w")
                    nc.vector.tensor_scalar(w[:, :nt], coord_ap, 54.0, RECIP,
                                            Op.add, Op.mult)
                    giq = pool.tile([128, CHUNK_TILES], i32, tag=tag + "i")
                    nc.vector.tensor_copy(giq[:, :nt], w[:, :nt])
                    gf = pool.tile([128, CHUNK_TILES], f32, tag=tag + "f")
                    nc.vector.tensor_copy(gf[:, :nt], giq[:, :nt])
                    d = pool.tile([128, CHUNK_TILES], f32, tag=tag + "d")
                    nc.vector.tensor_tensor(d[:, :nt], gf[:, :nt], w[:, :nt],
                                            Op.is_gt)
                    g = pool.tile([128, CHUNK_TILES], f32, tag=tag + "g")
                    nc.vector.tensor_tensor(g[:, :nt], gf[:, :nt], d[:, :nt],
                                            Op.subtract)
                    return g

                gx = floordiv(gt[:, 0:nt * 3:3], "gx")
                gy = floordiv(gt[:, 1:nt * 3:3], "gy")
                cell = pool.tile([128, CHUNK_TILES], f32, tag="cell")
                nc.vector.tensor_scalar(cell[:, :nt], gx[:, :nt], 360.0, None,
                                        Op.mult)
                nc.vector.tensor_tensor(cell[:, :nt], cell[:, :nt],
                                        gy[:, :nt], Op.add)
                nc.vector.tensor_scalar(cell[:, :nt], cell[:, :nt], 0.0,
                                        float(NCELL - 1), Op.max, Op.min)
                quad = pool.tile([128, CHUNK_TILES], f32, tag="quad")
                qi = pool.tile([128, CHUNK_TILES], i32, tag="qi")
                qtrue = pool.tile([128, CHUNK_TILES], f32, tag="qtrue")
                nc.vector.tensor_scalar(qtrue[:, :nt], cell[:, :nt], 0.25,
                                        None, Op.mult)
                nc.vector.tensor_copy(qi[:, :nt], qtrue[:, :nt])
                nc.vector.tensor_copy(quad[:, :nt], qi[:, :nt])
                qd = pool.tile([128, CHUNK_TILES], f32, tag="qd")
                nc.vector.tensor_tensor(qd[:, :nt], quad[:, :nt],
                                        qtrue[:, :nt], Op.is_gt)
                nc.vector.tensor_tensor(quad[:, :nt], quad[:, :nt],
                                        qd[:, :nt], Op.subtract)
                r4 = pool.tile([128, CHUNK_TILES], f32, tag="r4")
                nc.vector.tensor_scalar(r4[:, :nt], quad[:, :nt], -4.0, None,
                                        Op.mult)
                nc.vector.tensor_tensor(r4[:, :nt], r4[:, :nt], cell[:, :nt],
                                        Op.add)

                # ---- quad payload [tok, 4*64] bf16 ----
                pay = pool.tile([128, CHUNK_TILES * 4 * C], bf16, tag="pay")
                for t in range(nt):
                    m = pool.tile([128, 4 * C], bf16, tag="m256")
                    nc.vector.tensor_scalar(m[:], qiota[:], r4[:, t:t + 1],
                                            None, Op.is_equal)
                    p4 = pay[:, t * 4 * C:(t + 1) * 4 * C].rearrange(
                        "p (q c) -> p q c", c=C)
                    nc.vector.tensor_tensor(
                        p4, m[:].rearrange("p (q c) -> p q c", c=C),
                        xbf[:, t * C:(t + 1) * C].rearrange(
                            "p (u c) -> p u c", u=1).broadcast_to(
                                [128, 4, C]),
                        Op.mult)

                # ---- slab + ranks, per tile ----
                slab = epool.tile([128, CHUNK_TILES * 128 * D], bf16,
                                  tag="slab")
                slab4 = slab[:].rearrange("p (t q d) -> p t q d",
                                          q=128, d=D)
                rankc = pool.tile([128, CHUNK_TILES], f32, tag="rankc")

                for g4 in range((nt + 3) // 4):
                    n4 = min(4, nt - g4 * 4)
                    mt_g = []
                    for q in range(4):
                        mt_q = ppoolM.tile([64, 512], f32, tag=f"mt{q}")
                        mt_g.append(mt_q)
                    for ti in range(n4):
                        t = g4 * 4 + ti
                        # quad^T row for this tile
                        psTt = ppoolT.tile([128, 128], f32, tag="psTt")
                        psT = psTt[0:1, :]
                        nc.tensor.matmul(psT, quad[:, t:t + 1], ident[:])
                        qrow = pool.tile([1, 128], f32, tag="qrow")
                        nc.vector.tensor_copy(qrow[:], psT)
                        nqrow = pool.tile([1, 128], f32, tag="nqrow")
                        nc.vector.tensor_scalar(nqrow[:], psT, -1.0, None,
                                                Op.mult)
                        # E matrix: psD[i,j] = q_i - q_j ; E = (psD == 0)
                        psD = ppoolD.tile([128, 128], f32, tag="psD")
                        nc.tensor.matmul(psD[:], qrow[:], onesrow[:],
                                         start=True, stop=False)
                        nc.tensor.matmul(psD[:], onesrow[:], nqrow[:],
                                         start=False, stop=True)
                        E = epool.tile([128, 128], bf16, tag="E")
                        nc.vector.tensor_scalar(E[:], psD[:], 0.0, None,
                                                Op.is_equal)
                        # rank = count of earlier same-quad tokens
                        escr = pool.tile([128, 128], bf16, tag="escr")
                        nc.vector.tensor_tensor(escr[:], E[:], ltri[:],
                                                Op.mult)
                        nc.vector.tensor_reduce(rankc[:, t:t + 1], escr[:],
                                                AX.X, Op.add)
                        # merged^T per quarter: pay_q^T @ E  (E symmetric)
                        for q in range(4):
                            nc.tensor.matmul(
                                mt_g[q][:, ti * 128:(ti + 1) * 128],
                                pay[:, (t * 4 + q) * C:(t * 4 + q + 1) * C],
                                E[:])
                    # copy merged^T into slab positions
                    for q in range(4):
                        h, e = q >> 1, q & 1
                        nc.vector.tensor_copy(
                            slab4[64 * h:64 * h + 64,
                                  g4 * 4:g4 * 4 + n4, 0:128, e:e + 1],
                            mt_g[q][:, :n4 * 128].rearrange(
                                "p (t s) -> p t s ()", s=128))

                # ---- index selection: rank0 -> quad, else DUMP ----
                isz = pool.tile([128, CHUNK_TILES], f32, tag="isz")
                nc.vector.tensor_scalar(isz[:, :nt], rankc[:, :nt], 0.0,
                                        None, Op.is_equal)
                idxf = pool.tile([128, CHUNK_TILES], f32, tag="idxf")
                nc.vector.tensor_scalar(idxf[:, :nt], quad[:, :nt],
                                        float(DUMP), None, Op.subtract)
                nc.vector.tensor_tensor(idxf[:, :nt], idxf[:, :nt],
                                        isz[:, :nt], Op.mult)
                nc.vector.tensor_scalar(idxf[:, :nt], idxf[:, :nt],
                                        float(DUMP), None, Op.add)
                idx16 = pool.tile([128, CHUNK_TILES], i16, tag="idx16")
                nc.vector.tensor_copy(idx16[:, :nt], idxf[:, :nt])

                # ---- fold idx into wrapped layout [128, 8*nt] ----
                ncol = 8 * nt
                idxw = epool.tile([128, ncol_max], i16, tag="idxw")
                for j in range(8):
                    nc.sync.dma_start(
                        idxw[0:16, j:ncol:8],
                        idx16[16 * j:16 * j + 16, :nt])
                nc.sync.dma_start(idxw[16:32, :ncol], idxw[0:16, :ncol])
                nc.sync.dma_start(idxw[32:64, :ncol], idxw[0:32, :ncol])
                nc.sync.dma_start(idxw[64:128, :ncol], idxw[0:64, :ncol])

                # ---- scatter: one call per tile (call boundaries drain,
                # so only in-tile duplicates matter — and those are merged
                # and redirected to DUMP above) ----
                for t in range(nt):
                    nc.gpsimd.scatter_add(
                        A3,
                        idxw[:, 8 * t:8 * t + 8],
                        slab4[:, t, :, :],
                        128, NSLOT, D, 128)

                done += nt
                chunk_no += 1
                if chunk_no % per_flush == 0 or done >= ntiles:
                    nc.sync.dma_start(stage_d.ap()[flush_no], A[:])
                    if done < ntiles:
                        nc.vector.memset(A[:], 0.0)
                    flush_no += 1

            # ---- final f32 sum of the staged snapshots ----
            Wf = 1024
            per_part = NSLOT * D            # 64802
            off = 0
            while off < per_part:
                w = min(Wf, per_part - off)
                acc = pool.tile([128, Wf], f32, tag="facc")
                st0 = pool.tile([128, Wf], bf16, tag="fst0")
                nc.sync.dma_start(st0[:, :w], stage_d.ap()[0][:, off:off + w])
                nc.vector.tensor_copy(acc[:, :w], st0[:, :w])
                for f in range(1, flush_no):
                    stf = pool.tile([128, Wf], bf16, tag="fstf")
                    nc.sync.dma_start(stf[:, :w],
                                      stage_d.ap()[f][:, off:off + w])
                    nc.vector.tensor_tensor(acc[:, :w], acc[:, :w],
                                            stf[:, :w], Op.add)
                nc.sync.dma_start(out_d.ap()[:, off:off + w], acc[:, :w])
                off += w

    nc.compile()
    return nc


def _decode_grid(arr):
    """[128, NSLOT*D] f32 core dump -> [NCELL, C] f32 grid."""
    a = np.asarray(arr).astype(np.float32).reshape(128, NSLOT, D)
    a = a[:, :NQUAD, :]                       # drop dump slot
    g = a.reshape(2, C, NQUAD, D)             # [h, c, s, e]
    return g.transpose(2, 0, 3, 1).reshape(NCELL, C)


def kernel(geom_feats: np.ndarray, x: np.ndarray) -> np.ndarray:
    geom_feats = np.ascontiguousarray(geom_feats, dtype=np.float32)
    x = np.ascontiguousarray(x, dtype=np.float32)
    g2 = geom_feats.reshape(NP_TOTAL, 3)
    x2 = x.reshape(NP_TOTAL, C)

    if "nc" not in _cache:
        _cache["nc"] = build_program()
    nc = _cache["nc"]

    in_maps = []
    for c in range(NCORES):
        sl = slice(c * NP_CORE, (c + 1) * NP_CORE)
        in_maps.append({"geom": g2[sl], "x": x2[sl]})

    res = run_bass_kernel_spmd(nc, in_maps, core_ids=list(range(NCORES)))

    total = np.zeros((NCELL, C), np.float32)
    for c in range(NCORES):
        total += _decode_grid(res.results[c]["out"])
    out = total.reshape(H, W, C).transpose(2, 0, 1)[None]
    return np.ascontiguousarray(out)
